# revision 28
# baseline (speedup 1.0000x reference)
"""CategorySpecificLinear Trainium2 kernel (v3: bf16 weight-stationary).

out[t] = x[t] @ weight[category_id[t]] + bias[category_id[t]]

Strategy: expert-parallel over the 8 categories (C == n_cores == 8).
Host routes tokens by category; core c computes its category's tokens.

Device-side formulation (per core, transposed output):
    outT[o, t] = sum_k wT[k, o] * xT[k, t] + bias[o]
with the weight tile [128k x 128o] STATIONARY in the PE array and x
streamed as the moving operand, so PE stream cycles = OT*KO*T_pad
(~34k cycles ~ 14.1 us warm @2.4GHz) with zero m-tile quantization
waste. All operands bf16 (psum accumulates fp32) -> half the HBM
traffic of fp32 and 1 col/cycle warm on the PE.

Schedule (8 psum banks = 4 o-tiles x 2 T-halves in flight):
  phase 0 (o-tiles 0-3): k-OUTER -- each k-step needs only x[k] and
    w[g0,k], so the PE starts ~1.5 us after the first small chunks
    land instead of after the full 3 MB input load.
  phase 1 (o-tiles 4-7): everything is SBUF-resident by now, so run
    ot-OUTER: each o-tile's output drains (DVE/ACT bias-add + bf16
    store) while the next o-tile computes -> only the last o-tile's
    add+store is kernel tail.
Warm-up: ~24 dummy matmuls bridge the initial DMA wait so the HAM
clock gate reaches 8/8 before the real stream.

DMA: three queues (sync + scalar HWDGE, gpsimd SWDGE) each see
~190 GB/s when all active and lose ~1 us between chained DMAs, so
inputs are split in 2 chunks per queue (small first chunk for early
PE start): sync = x, scalar = w[o-tiles 0-3], gpsimd = bias + w[o-
tiles 4-7]. Host pre-arranges x as [p][k][t] and w as [p][g][k][o]
(per-partition contiguous) so every load is a plain 2D slice.
Per-core HBM: x 1.06 + w 2 + out 1.06 MB = 4.2 MB.

bf16 numerics: rel err ~3e-3 on dot-1024 (gate is 2e-2).
"""

import contextlib
import ctypes
import os
import sys
import types

import numpy as np

sys.path.insert(0, "/opt/trn_rl_repo")


def _ensure_ntff_hook():
    """Provide antenv.axon_hooks if the image lacks it.

    concourse.bass_utils imports antenv.axon_hooks.get_axon_ntff_profile_hook
    when trace=True under axon; some agent images don't ship that module, in
    which case the boot's NTFF hook registration silently degrades and the
    import in bass_utils crashes. Recreate the slim ctypes hook here
    (mirrors trn_agent_boot.trn_boot._ntff_profile_via_ctypes).
    """
    try:
        import antenv.axon_hooks  # noqa: F401

        return
    except ImportError:
        pass

    so_path = "/opt/axon/libaxon_pjrt.so"
    hook = None
    if os.path.exists(so_path):
        lib = ctypes.CDLL(so_path)
        if hasattr(lib, "axon_start_nrt_profile"):
            lib.axon_start_nrt_profile.argtypes = [
                ctypes.POINTER(ctypes.c_int64),
                ctypes.c_size_t,
            ]
            lib.axon_start_nrt_profile.restype = ctypes.c_int64
            lib.axon_stop_nrt_profile.argtypes = [ctypes.c_char_p]
            lib.axon_stop_nrt_profile.restype = ctypes.c_int64

            @contextlib.contextmanager
            def hook(output_dir, device_ids):
                import jax

                jax.devices()
                if device_ids:
                    ids = (ctypes.c_int64 * len(device_ids))(*device_ids)
                    rc = lib.axon_start_nrt_profile(ids, len(device_ids))
                else:
                    rc = lib.axon_start_nrt_profile(None, 0)
                if rc != 0:
                    raise RuntimeError(f"axon_start_nrt_profile rc={rc}")
                try:
                    yield
                finally:
                    n = lib.axon_stop_nrt_profile(str(output_dir).encode())
                    if n <= 0:
                        print(
                            f"ntff profile: rc={n} writing {output_dir}",
                            file=sys.stderr,
                        )

    mod = types.ModuleType("antenv.axon_hooks")
    _state = {"hook": hook}
    mod.set_axon_ntff_profile_hook = lambda h: _state.__setitem__("hook", h)
    mod.get_axon_ntff_profile_hook = lambda: _state["hook"]
    sys.modules["antenv.axon_hooks"] = mod
    try:
        import antenv

        antenv.axon_hooks = mod
    except ImportError:
        pass


_ensure_ntff_hook()

import ml_dtypes

import concourse.bass as bass
import concourse.bacc as bacc_mod
import concourse.mybir as mybir
import concourse.tile as tile
from concourse.bass_utils import run_bass_kernel_spmd

N_CORES = 8
P = 128
BF16 = np.dtype(ml_dtypes.bfloat16)

_nc_cache = {}
LAST_RESULTS = None  # BassKernelResults of the most recent run (for test.py)

N_WARM = 30  # dummy matmuls bridging the initial DMA wait (HAM warm-up)


def _build_nc(T_pad: int, D: int, O: int):
    KO = D // P
    OT = O // P
    assert KO == 8 and OT == 8
    bf = mybir.dt.bfloat16
    f32 = mybir.dt.float32

    # moving-operand split: one psum bank holds <=512 fp32 per partition,
    # so stream T in two pieces. Asymmetric (big A, 128-wide B) so the
    # kernel tail -- the last o-tile's B bias-add + store -- is small;
    # total stream cycles are unchanged.
    if T_pad <= 512:
        TA, TB = T_pad, 0
    elif T_pad <= 512 + 64:
        TA, TB = T_pad - 64, 64
    elif T_pad <= 512 + 128:
        TA, TB = T_pad - 128, 128
    else:
        TA = -(-(T_pad // 2) // 16) * 16
        TB = T_pad - TA
        assert TB <= 512

    S = T_pad + 4 * P  # per-k elems per partition in xw (x slice + 4 o-tiles of w)
    GW = 4 * P  # per-k elems per partition in w1 (512)

    nc = bacc_mod.Bacc()
    xw = nc.dram_tensor("xw", [P, KO * S], bf, kind="ExternalInput")
    w1 = nc.dram_tensor("w1", [P, KO * GW], bf, kind="ExternalInput")
    bias = nc.dram_tensor("bias", [P, OT], f32, kind="ExternalInput")
    out = nc.dram_tensor("out", [O, T_pad], bf, kind="ExternalOutput")

    with tile.TileContext(nc) as tc:
        with (
            tc.tile_pool(name="resident", bufs=1) as rpool,
            tc.tile_pool(name="psum", bufs=1, space="PSUM") as psum_pool,
            tc.tile_pool(name="obuf", bufs=8) as opool,
        ):
            # ---- input DMAs -------------------------------------------------
            # The DMA rings round-robin fairly at packet granularity, so
            # ring FIFO order is the only priority mechanism. Phase 0's
            # data is ONE combined per-k stream (x slice + its 4 weight
            # tiles, exactly the PE's per-k consumption) alternated
            # across both HWDGE rings; k0/k1 are split x-vs-w across the
            # rings so the pipeline fills fast. The phase-1 weights ride
            # the same rings' tails in k order, so they never steal
            # bandwidth from the k-paced phase-0 stream. gpsimd carries
            # bias + the non-critical output stores.
            # memset first so it's the gpsimd engine's first instruction:
            # the dummy-matmul warm-up stream can then start ~1us earlier
            warm_sb = rpool.tile([P, P], bf, tag="warm")
            nc.gpsimd.memset(warm_sb[:], 0.0)

            xw_sb = rpool.tile([P, KO * S], bf, tag="xw")

            def ldxw(eng, lo, hi):
                eng.dma_start(xw_sb[:, lo:hi], xw[:, lo:hi])

            ldxw(nc.sync, 0, T_pad)  # x k0
            ldxw(nc.scalar, T_pad, S)  # w k0
            ldxw(nc.gpsimd, S, 2 * S)  # k1 block rides the third (SWDGE) path
            for k in range(2, KO):
                ldxw(nc.sync if k % 2 == 0 else nc.scalar, k * S, (k + 1) * S)

            w1_sb = rpool.tile([P, KO * GW], bf, tag="w1")
            nc.sync.dma_start(w1_sb[:, : 4 * GW], w1[:, : 4 * GW])
            nc.scalar.dma_start(w1_sb[:, 4 * GW :], w1[:, 4 * GW :])

            bias_sb = rpool.tile([P, OT], f32, tag="bias")
            nc.gpsimd.dma_start(bias_sb[:], bias[:, :])

            def wsl(k, ot):
                if ot < 4:
                    base = k * S + T_pad + ot * P
                    return xw_sb[:, base : base + P]
                base = k * GW + (ot - 4) * P
                return w1_sb[:, base : base + P]

            def xsl(k, lo, hi):
                return xw_sb[:, k * S + lo : k * S + hi]

            def emit_out(ot, psA, psB):
                o_sb = opool.tile([P, T_pad], bf, tag="ot", name=f"o{ot}")
                bcol = bias_sb[:, ot : ot + 1]
                nc.vector.tensor_scalar_add(o_sb[:, :TA], psA[:], bcol)
                if TB:
                    nc.scalar.activation(
                        o_sb[:, TA:],
                        psB[:],
                        mybir.ActivationFunctionType.Identity,
                        bias=bcol,
                        scale=1.0,
                    )
                eng = nc.sync if ot % 2 == 0 else nc.scalar
                eng.dma_start(out[ot * P : (ot + 1) * P, :], o_sb[:])

            def mk_psum(i, nm):
                psA = psum_pool.tile([P, TA], f32, tag=f"psA{i}", name=f"psA{nm}")
                psB = (
                    psum_pool.tile([P, TB], f32, tag=f"psB{i}", name=f"psB{nm}")
                    if TB
                    else None
                )
                return psA, psB

            # ---- phase 0: o-tiles 0-3, k-outer (DMA-paced) ------------------
            ps = [mk_psum(i, f"p0_{i}") for i in range(4)]

            for _ in range(N_WARM):
                nc.tensor.matmul(
                    ps[0][0][:, :P],
                    lhsT=warm_sb[:],
                    rhs=warm_sb[:],
                    start=True,
                    stop=True,
                )

            for k in range(KO):
                for i in range(4):
                    lhsT = wsl(k, i)
                    nc.tensor.matmul(
                        ps[i][0][:],
                        lhsT=lhsT,
                        rhs=xsl(k, 0, TA),
                        start=(k == 0),
                        stop=(k == KO - 1),
                    )
                    if TB:
                        nc.tensor.matmul(
                            ps[i][1][:],
                            lhsT=lhsT,
                            rhs=xsl(k, TA, T_pad),
                            start=(k == 0),
                            stop=(k == KO - 1),
                        )
            for i in range(4):
                emit_out(i, ps[i][0], ps[i][1])

            # ---- phase 1: o-tiles 4-7, ot-outer (SBUF-resident) -------------
            # Per-half psum groups: the A-half's bias-add runs on DVE while
            # the B-half's matmuls stream, so only the final half's add +
            # store is kernel tail. The last o-tile's store is split across
            # both HWDGE queues to halve its wire+issue time.
            for i in range(4):
                ot = 4 + i
                psA, psB = mk_psum(i, f"p1_{i}")
                o_sb = opool.tile([P, T_pad], bf, tag="ot", name=f"o{ot}")
                bcol = bias_sb[:, ot : ot + 1]
                last = ot == OT - 1
                orow = out[ot * P : (ot + 1) * P, :]
                for k in range(KO):
                    nc.tensor.matmul(
                        psA[:],
                        lhsT=wsl(k, ot),
                        rhs=xsl(k, 0, TA),
                        start=(k == 0),
                        stop=(k == KO - 1),
                    )
                nc.vector.tensor_scalar_add(o_sb[:, :TA], psA[:], bcol)
                if last and TB:
                    # critical tail: A half stores while the B half's
                    # matmuls stream; B's bias-add is split DVE/ACT and
                    # its store rides the other (idle) HWDGE ring
                    nc.sync.dma_start(orow[:, :TA], o_sb[:, :TA])
                if TB:
                    for k in range(KO):
                        nc.tensor.matmul(
                            psB[:],
                            lhsT=wsl(k, ot),
                            rhs=xsl(k, TA, T_pad),
                            start=(k == 0),
                            stop=(k == KO - 1),
                        )
                    nc.scalar.activation(
                        o_sb[:, TA:],
                        psB[:],
                        mybir.ActivationFunctionType.Identity,
                        bias=bcol,
                        scale=1.0,
                    )
                    if last:
                        nc.scalar.dma_start(orow[:, TA:], o_sb[:, TA:])
                if not last:
                    eng = nc.sync if ot % 2 == 0 else nc.scalar
                    eng.dma_start(orow, o_sb[:])
                elif not TB:
                    nc.sync.dma_start(orow, o_sb[:])
    nc.finalize()
    return nc


def kernel(x, category_id, weight, bias):
    global LAST_RESULTS
    x = np.asarray(x)
    category_id = np.asarray(category_id)
    weight = np.ascontiguousarray(np.asarray(weight), dtype=np.float32)
    bias = np.ascontiguousarray(np.asarray(bias), dtype=np.float32)

    orig_shape = x.shape
    D = orig_shape[-1]
    C, _, O = weight.shape
    KO, OT = D // P, O // P
    assert C == N_CORES and KO == 8 and OT == 8

    T = int(np.prod(orig_shape[:-1]))
    x_flat = np.ascontiguousarray(x.reshape(T, D), dtype=np.float32)
    cid = category_id.reshape(T).astype(np.int64)

    idx_per_c = [np.flatnonzero(cid == c) for c in range(C)]
    counts = [len(ix) for ix in idx_per_c]
    T_pad = max(32, -(-max(counts) // 16) * 16)

    key = (T_pad, D, O)
    if key not in _nc_cache:
        _nc_cache[key] = _build_nc(T_pad, D, O)
    nc = _nc_cache[key]

    # pre-arranged per-partition-contiguous layouts (see _build_nc)
    in_maps = []
    for c in range(C):
        xc = np.zeros((T_pad, D), dtype=np.float32)
        xc[: counts[c]] = x_flat[idx_per_c[c]]
        # [t, (k p)] -> [p, k, t]
        xh = xc.T.reshape(KO, P, T_pad).transpose(1, 0, 2).astype(BF16)
        # [(k p), (g i o)] -> [g, p, k, i*o]   (g = ot//4, i = ot%4)
        wh = (
            weight[c]
            .reshape(KO, P, 2, 4 * P)
            .transpose(2, 1, 0, 3)
            .astype(BF16)
        )
        # combined per-k stream for phase 0: [p][k][x_t | w_g0]
        xwh = np.ascontiguousarray(
            np.concatenate([xh, wh[0]], axis=2)
        ).reshape(P, KO * (T_pad + 4 * P))
        w1h = np.ascontiguousarray(wh[1]).reshape(P, KO * 4 * P)
        # [(ot o)] -> [o, ot]
        bh = np.ascontiguousarray(bias[c].reshape(OT, P).T)
        in_maps.append({"xw": xwh, "w1": w1h, "bias": bh})

    res = run_bass_kernel_spmd(nc, in_maps, list(range(N_CORES)))
    LAST_RESULTS = res

    out_flat = np.empty((T, O), dtype=np.float32)
    for c in range(C):
        oc = np.asarray(res.results[c]["out"])  # [O, T_pad] bf16
        out_flat[idx_per_c[c]] = oc[:, : counts[c]].T.astype(np.float32)
    return out_flat.reshape(*orig_shape[:-1], O)


# revision 29
# speedup vs baseline: 1.0048x; 1.0048x over previous
"""CategorySpecificLinear Trainium2 kernel (v3: bf16 weight-stationary).

out[t] = x[t] @ weight[category_id[t]] + bias[category_id[t]]

Strategy: expert-parallel over the 8 categories (C == n_cores == 8).
Host routes tokens by category; core c computes its category's tokens.

Device-side formulation (per core, transposed output):
    outT[o, t] = sum_k wT[k, o] * xT[k, t] + bias[o]
with the weight tile [128k x 128o] STATIONARY in the PE array and x
streamed as the moving operand, so PE stream cycles = OT*KO*T_pad
(~34k cycles ~ 14.1 us warm @2.4GHz) with zero m-tile quantization
waste. All operands bf16 (psum accumulates fp32) -> half the HBM
traffic of fp32 and 1 col/cycle warm on the PE.

Schedule (8 psum banks = 4 o-tiles x 2 T-halves in flight):
  phase 0 (o-tiles 0-3): k-OUTER -- each k-step needs only x[k] and
    w[g0,k], so the PE starts ~1.5 us after the first small chunks
    land instead of after the full 3 MB input load.
  phase 1 (o-tiles 4-7): everything is SBUF-resident by now, so run
    ot-OUTER: each o-tile's output drains (DVE/ACT bias-add + bf16
    store) while the next o-tile computes -> only the last o-tile's
    add+store is kernel tail.
Warm-up: ~24 dummy matmuls bridge the initial DMA wait so the HAM
clock gate reaches 8/8 before the real stream.

DMA: three queues (sync + scalar HWDGE, gpsimd SWDGE) each see
~190 GB/s when all active and lose ~1 us between chained DMAs, so
inputs are split in 2 chunks per queue (small first chunk for early
PE start): sync = x, scalar = w[o-tiles 0-3], gpsimd = bias + w[o-
tiles 4-7]. Host pre-arranges x as [p][k][t] and w as [p][g][k][o]
(per-partition contiguous) so every load is a plain 2D slice.
Per-core HBM: x 1.06 + w 2 + out 1.06 MB = 4.2 MB.

bf16 numerics: rel err ~3e-3 on dot-1024 (gate is 2e-2).
"""

import contextlib
import ctypes
import os
import sys
import types

import numpy as np

sys.path.insert(0, "/opt/trn_rl_repo")


def _ensure_ntff_hook():
    """Provide antenv.axon_hooks if the image lacks it.

    concourse.bass_utils imports antenv.axon_hooks.get_axon_ntff_profile_hook
    when trace=True under axon; some agent images don't ship that module, in
    which case the boot's NTFF hook registration silently degrades and the
    import in bass_utils crashes. Recreate the slim ctypes hook here
    (mirrors trn_agent_boot.trn_boot._ntff_profile_via_ctypes).
    """
    try:
        import antenv.axon_hooks  # noqa: F401

        return
    except ImportError:
        pass

    so_path = "/opt/axon/libaxon_pjrt.so"
    hook = None
    if os.path.exists(so_path):
        lib = ctypes.CDLL(so_path)
        if hasattr(lib, "axon_start_nrt_profile"):
            lib.axon_start_nrt_profile.argtypes = [
                ctypes.POINTER(ctypes.c_int64),
                ctypes.c_size_t,
            ]
            lib.axon_start_nrt_profile.restype = ctypes.c_int64
            lib.axon_stop_nrt_profile.argtypes = [ctypes.c_char_p]
            lib.axon_stop_nrt_profile.restype = ctypes.c_int64

            @contextlib.contextmanager
            def hook(output_dir, device_ids):
                import jax

                jax.devices()
                if device_ids:
                    ids = (ctypes.c_int64 * len(device_ids))(*device_ids)
                    rc = lib.axon_start_nrt_profile(ids, len(device_ids))
                else:
                    rc = lib.axon_start_nrt_profile(None, 0)
                if rc != 0:
                    raise RuntimeError(f"axon_start_nrt_profile rc={rc}")
                try:
                    yield
                finally:
                    n = lib.axon_stop_nrt_profile(str(output_dir).encode())
                    if n <= 0:
                        print(
                            f"ntff profile: rc={n} writing {output_dir}",
                            file=sys.stderr,
                        )

    mod = types.ModuleType("antenv.axon_hooks")
    _state = {"hook": hook}
    mod.set_axon_ntff_profile_hook = lambda h: _state.__setitem__("hook", h)
    mod.get_axon_ntff_profile_hook = lambda: _state["hook"]
    sys.modules["antenv.axon_hooks"] = mod
    try:
        import antenv

        antenv.axon_hooks = mod
    except ImportError:
        pass


_ensure_ntff_hook()

import ml_dtypes

import concourse.bass as bass
import concourse.bacc as bacc_mod
import concourse.mybir as mybir
import concourse.tile as tile
from concourse.bass_utils import run_bass_kernel_spmd

N_CORES = 8
P = 128
BF16 = np.dtype(ml_dtypes.bfloat16)

_nc_cache = {}
LAST_RESULTS = None  # BassKernelResults of the most recent run (for test.py)

N_WARM = 30  # dummy matmuls bridging the initial DMA wait (HAM warm-up)


def _build_nc(T_pad: int, D: int, O: int):
    KO = D // P
    OT = O // P
    assert KO == 8 and OT == 8
    bf = mybir.dt.bfloat16
    f32 = mybir.dt.float32

    # moving-operand split: one psum bank holds <=512 fp32 per partition,
    # so stream T in two pieces. Asymmetric (big A, 128-wide B) so the
    # kernel tail -- the last o-tile's B bias-add + store -- is small;
    # total stream cycles are unchanged.
    if T_pad <= 512:
        TA, TB = T_pad, 0
    elif T_pad <= 512 + 128:
        TA, TB = T_pad - 128, 128
    else:
        TA = -(-(T_pad // 2) // 16) * 16
        TB = T_pad - TA
        assert TB <= 512

    S = T_pad + 4 * P  # per-k elems per partition in xw (x slice + 4 o-tiles of w)
    GW = 4 * P  # per-k elems per partition in w1 (512)

    nc = bacc_mod.Bacc()
    xw = nc.dram_tensor("xw", [P, KO * S], bf, kind="ExternalInput")
    w1 = nc.dram_tensor("w1", [P, KO * GW], bf, kind="ExternalInput")
    bias = nc.dram_tensor("bias", [P, OT], f32, kind="ExternalInput")
    out = nc.dram_tensor("out", [O, T_pad], bf, kind="ExternalOutput")

    with tile.TileContext(nc) as tc:
        with (
            tc.tile_pool(name="resident", bufs=1) as rpool,
            tc.tile_pool(name="psum", bufs=1, space="PSUM") as psum_pool,
            tc.tile_pool(name="obuf", bufs=8) as opool,
        ):
            # ---- input DMAs -------------------------------------------------
            # The DMA rings round-robin fairly at packet granularity, so
            # ring FIFO order is the only priority mechanism. Phase 0's
            # data is ONE combined per-k stream (x slice + its 4 weight
            # tiles, exactly the PE's per-k consumption) alternated
            # across both HWDGE rings; k0/k1 are split x-vs-w across the
            # rings so the pipeline fills fast. The phase-1 weights ride
            # the same rings' tails in k order, so they never steal
            # bandwidth from the k-paced phase-0 stream. gpsimd carries
            # bias + the non-critical output stores.
            # memset first so it's the gpsimd engine's first instruction:
            # the dummy-matmul warm-up stream can then start ~1us earlier
            warm_sb = rpool.tile([P, P], bf, tag="warm")
            nc.gpsimd.memset(warm_sb[:], 0.0)

            xw_sb = rpool.tile([P, KO * S], bf, tag="xw")

            def ldxw(eng, lo, hi):
                eng.dma_start(xw_sb[:, lo:hi], xw[:, lo:hi])

            ldxw(nc.sync, 0, T_pad)  # x k0
            ldxw(nc.scalar, T_pad, S)  # w k0
            ldxw(nc.gpsimd, S, 2 * S)  # k1 block rides the third (SWDGE) path
            for k in range(2, KO):
                ldxw(nc.sync if k % 2 == 0 else nc.scalar, k * S, (k + 1) * S)

            w1_sb = rpool.tile([P, KO * GW], bf, tag="w1")
            nc.sync.dma_start(w1_sb[:, : 4 * GW], w1[:, : 4 * GW])
            nc.scalar.dma_start(w1_sb[:, 4 * GW :], w1[:, 4 * GW :])

            bias_sb = rpool.tile([P, OT], f32, tag="bias")
            nc.gpsimd.dma_start(bias_sb[:], bias[:, :])

            def wsl(k, ot):
                if ot < 4:
                    base = k * S + T_pad + ot * P
                    return xw_sb[:, base : base + P]
                base = k * GW + (ot - 4) * P
                return w1_sb[:, base : base + P]

            def xsl(k, lo, hi):
                return xw_sb[:, k * S + lo : k * S + hi]

            def emit_out(ot, psA, psB):
                o_sb = opool.tile([P, T_pad], bf, tag="ot", name=f"o{ot}")
                bcol = bias_sb[:, ot : ot + 1]
                nc.vector.tensor_scalar_add(o_sb[:, :TA], psA[:], bcol)
                if TB:
                    nc.scalar.activation(
                        o_sb[:, TA:],
                        psB[:],
                        mybir.ActivationFunctionType.Identity,
                        bias=bcol,
                        scale=1.0,
                    )
                eng = nc.sync if ot % 2 == 0 else nc.scalar
                eng.dma_start(out[ot * P : (ot + 1) * P, :], o_sb[:])

            def mk_psum(i, nm):
                psA = psum_pool.tile([P, TA], f32, tag=f"psA{i}", name=f"psA{nm}")
                psB = (
                    psum_pool.tile([P, TB], f32, tag=f"psB{i}", name=f"psB{nm}")
                    if TB
                    else None
                )
                return psA, psB

            # ---- phase 0: o-tiles 0-3, k-outer (DMA-paced) ------------------
            ps = [mk_psum(i, f"p0_{i}") for i in range(4)]

            for _ in range(N_WARM):
                nc.tensor.matmul(
                    ps[0][0][:, :P],
                    lhsT=warm_sb[:],
                    rhs=warm_sb[:],
                    start=True,
                    stop=True,
                )

            for k in range(KO):
                for i in range(4):
                    lhsT = wsl(k, i)
                    nc.tensor.matmul(
                        ps[i][0][:],
                        lhsT=lhsT,
                        rhs=xsl(k, 0, TA),
                        start=(k == 0),
                        stop=(k == KO - 1),
                    )
                    if TB:
                        nc.tensor.matmul(
                            ps[i][1][:],
                            lhsT=lhsT,
                            rhs=xsl(k, TA, T_pad),
                            start=(k == 0),
                            stop=(k == KO - 1),
                        )
            for i in range(4):
                emit_out(i, ps[i][0], ps[i][1])

            # ---- phase 1: o-tiles 4-7, ot-outer (SBUF-resident) -------------
            # Per-half psum groups: the A-half's bias-add runs on DVE while
            # the B-half's matmuls stream, so only the final half's add +
            # store is kernel tail. The last o-tile's store is split across
            # both HWDGE queues to halve its wire+issue time.
            for i in range(4):
                ot = 4 + i
                psA, psB = mk_psum(i, f"p1_{i}")
                o_sb = opool.tile([P, T_pad], bf, tag="ot", name=f"o{ot}")
                bcol = bias_sb[:, ot : ot + 1]
                last = ot == OT - 1
                orow = out[ot * P : (ot + 1) * P, :]
                for k in range(KO):
                    nc.tensor.matmul(
                        psA[:],
                        lhsT=wsl(k, ot),
                        rhs=xsl(k, 0, TA),
                        start=(k == 0),
                        stop=(k == KO - 1),
                    )
                nc.vector.tensor_scalar_add(o_sb[:, :TA], psA[:], bcol)
                if last and TB:
                    # critical tail: A half stores while the B half's
                    # matmuls stream; B's bias-add is split DVE/ACT and
                    # its store rides the other (idle) HWDGE ring
                    nc.sync.dma_start(orow[:, :TA], o_sb[:, :TA])
                if TB:
                    for k in range(KO):
                        nc.tensor.matmul(
                            psB[:],
                            lhsT=wsl(k, ot),
                            rhs=xsl(k, TA, T_pad),
                            start=(k == 0),
                            stop=(k == KO - 1),
                        )
                    nc.scalar.activation(
                        o_sb[:, TA:],
                        psB[:],
                        mybir.ActivationFunctionType.Identity,
                        bias=bcol,
                        scale=1.0,
                    )
                    if last:
                        nc.scalar.dma_start(orow[:, TA:], o_sb[:, TA:])
                if not last:
                    eng = nc.sync if ot % 2 == 0 else nc.scalar
                    eng.dma_start(orow, o_sb[:])
                elif not TB:
                    nc.sync.dma_start(orow, o_sb[:])
    nc.finalize()
    return nc


def kernel(x, category_id, weight, bias):
    global LAST_RESULTS
    x = np.asarray(x)
    category_id = np.asarray(category_id)
    weight = np.ascontiguousarray(np.asarray(weight), dtype=np.float32)
    bias = np.ascontiguousarray(np.asarray(bias), dtype=np.float32)

    orig_shape = x.shape
    D = orig_shape[-1]
    C, _, O = weight.shape
    KO, OT = D // P, O // P
    assert C == N_CORES and KO == 8 and OT == 8

    T = int(np.prod(orig_shape[:-1]))
    x_flat = np.ascontiguousarray(x.reshape(T, D), dtype=np.float32)
    cid = category_id.reshape(T).astype(np.int64)

    idx_per_c = [np.flatnonzero(cid == c) for c in range(C)]
    counts = [len(ix) for ix in idx_per_c]
    T_pad = max(32, -(-max(counts) // 16) * 16)

    key = (T_pad, D, O)
    if key not in _nc_cache:
        _nc_cache[key] = _build_nc(T_pad, D, O)
    nc = _nc_cache[key]

    # pre-arranged per-partition-contiguous layouts (see _build_nc)
    in_maps = []
    for c in range(C):
        xc = np.zeros((T_pad, D), dtype=np.float32)
        xc[: counts[c]] = x_flat[idx_per_c[c]]
        # [t, (k p)] -> [p, k, t]
        xh = xc.T.reshape(KO, P, T_pad).transpose(1, 0, 2).astype(BF16)
        # [(k p), (g i o)] -> [g, p, k, i*o]   (g = ot//4, i = ot%4)
        wh = (
            weight[c]
            .reshape(KO, P, 2, 4 * P)
            .transpose(2, 1, 0, 3)
            .astype(BF16)
        )
        # combined per-k stream for phase 0: [p][k][x_t | w_g0]
        xwh = np.ascontiguousarray(
            np.concatenate([xh, wh[0]], axis=2)
        ).reshape(P, KO * (T_pad + 4 * P))
        w1h = np.ascontiguousarray(wh[1]).reshape(P, KO * 4 * P)
        # [(ot o)] -> [o, ot]
        bh = np.ascontiguousarray(bias[c].reshape(OT, P).T)
        in_maps.append({"xw": xwh, "w1": w1h, "bias": bh})

    res = run_bass_kernel_spmd(nc, in_maps, list(range(N_CORES)))
    LAST_RESULTS = res

    out_flat = np.empty((T, O), dtype=np.float32)
    for c in range(C):
        oc = np.asarray(res.results[c]["out"])  # [O, T_pad] bf16
        out_flat[idx_per_c[c]] = oc[:, : counts[c]].T.astype(np.float32)
    return out_flat.reshape(*orig_shape[:-1], O)
